# revision 1
# baseline (speedup 1.0000x reference)
"""CrystalGraphConvNet on 8 trn2 NeuronCores (Bass/Tile), self-contained.

Atoms sharded 12500/core. Neighbor gather: [P,1] indirect DMA rows from an
AllGathered bf16 atom-feature table. BN stats: PSUM ones-matmuls + tiny
AllReduce. Crystal mean-pool: is_equal assignment matmul into a 512-wide
local crystal window, AllGather + shifted adds to rebuild global sums.
"""
import os

import numpy as np
import ml_dtypes

import concourse.bass as bass
import concourse.mybir as mybir
import concourse.tile as tile
from concourse.bass import IndirectOffsetOnAxis
from concourse.bass_utils import run_bass_kernel_spmd
from concourse.masks import make_identity

F32 = mybir.dt.float32
BF16 = mybir.dt.bfloat16
I32 = mybir.dt.int32
AF = mybir.ActivationFunctionType
ALU = mybir.AluOpType

N, M, ORIG, NBR, F, H, N0 = 100000, 12, 92, 41, 64, 128, 2000
NC = 8
S = N // NC
EPS = 1e-5
TILES = [(t * 128, min(128, S - t * 128)) for t in range((S + 127) // 128)]
NT = len(TILES)
CLOC = 512
CB = N0 // NC  # 250
BASES = [max(0, k * CB - 128) for k in range(NC)]
N0P = 2560

_ctr = [0]


def split_sync_waits(nc, limit=1):
    f = nc.m.functions[0]
    for b in f.blocks:
        new, changed = [], False
        for i in b.instructions:
            si = i.sync_info
            if si is not None and len(si.on_wait) > limit:
                waits = list(si.on_wait)
                head, rest = waits[:-limit], waits[-limit:]
                for k in range(0, len(head), limit):
                    _ctr[0] += 1
                    d = mybir.InstNoOp(name=f"waitsplit-{_ctr[0]}", ins=[], outs=[])
                    d.engine = i.engine
                    d.sync_info = mybir.SyncInfo(on_wait=head[k:k + limit], on_update=[])
                    new.append(d)
                si.on_wait = rest
                changed = True
            new.append(i)
        if changed:
            b.instructions = new


def build_program():
    nc = bass.Bass(num_devices=NC)

    atomT = nc.dram_tensor("atomT", [ORIG + 1, S], F32, kind="ExternalInput")
    nbrT = nc.dram_tensor("nbrT", [NT, NBR, M * 128], BF16, kind="ExternalInput")
    idx = nc.dram_tensor("idx", [S, M], I32, kind="ExternalInput")
    cidl = nc.dram_tensor("cidl", [S, 1], I32, kind="ExternalInput")
    invc = nc.dram_tensor("invc", [1, N0P], F32, kind="ExternalInput")
    wemb = nc.dram_tensor("wemb", [ORIG + 1, F], F32, kind="ExternalInput")
    wself = nc.dram_tensor("wself", [2, F + 1, 2 * F], F32, kind="ExternalInput")
    wnbr = nc.dram_tensor("wnbr", [2, F, 2 * F], BF16, kind="ExternalInput")
    wnbrf = nc.dram_tensor("wnbrf", [2, NBR, 2 * F], BF16, kind="ExternalInput")
    bn1g = nc.dram_tensor("bn1g", [2, 1, 2 * F], F32, kind="ExternalInput")
    bn1b = nc.dram_tensor("bn1b", [2, 1, 2 * F], F32, kind="ExternalInput")
    bn2g = nc.dram_tensor("bn2g", [2, 1, F], F32, kind="ExternalInput")
    bn2b = nc.dram_tensor("bn2b", [2, 1, F], F32, kind="ExternalInput")
    wfc = nc.dram_tensor("wfc", [F + 1, H], F32, kind="ExternalInput")
    wout = nc.dram_tensor("wout", [H, 1], F32, kind="ExternalInput")
    bout = nc.dram_tensor("bout", [1, 1], F32, kind="ExternalInput")

    out = nc.dram_tensor("out", [1, N0], F32, kind="ExternalOutput")

    ag_in = nc.dram_tensor("ag_in", [S, 2 * F], BF16)
    a_full = nc.dram_tensor("a_full", [N, 2 * F], BF16, addr_space="Shared")
    stash = nc.dram_tensor("stash", [NT, 128, M * 2 * F], BF16)
    cc_in = nc.dram_tensor("cc_in", [1, 1024], F32)
    cc_out = nc.dram_tensor("cc_out", [1, 1024], F32, addr_space="Shared")
    cc2_in = nc.dram_tensor("cc2_in", [1, 2 * F], F32)
    cc2_out = nc.dram_tensor("cc2_out", [1, 2 * F], F32, addr_space="Shared")
    cr_in = nc.dram_tensor("cr_in", [F, CLOC], F32)
    cr_out = nc.dram_tensor("cr_out", [NC, F, CLOC], F32, addr_space="Shared")

    RG = [list(range(NC))]
    NQ = M * 2 * F // 512  # 3

    with tile.TileContext(nc) as tc:
        with tc.tile_pool(name="persist", bufs=1) as pp, \
             tc.tile_pool(name="work", bufs=2) as wp, \
             tc.tile_pool(name="small", bufs=1) as sp:

            ident_b = pp.tile([128, 128], BF16)
            make_identity(nc, ident_b[:])
            ones_col = pp.tile([128, 1], F32)
            nc.vector.memset(ones_col[:], 1.0)
            ones_col_b = pp.tile([128, 1], BF16)
            nc.vector.memset(ones_col_b[:], 1.0)
            ones_row = pp.tile([1, 128], F32)
            nc.vector.memset(ones_row[:], 1.0)

            a_sb = pp.tile([128, NT * F], F32, tag="a_sb")
            a1_sb = pp.tile([128, NT * F], F32, tag="a1_sb")
            sum_sb = pp.tile([128, NT * F], F32, tag="sum_sb")
            s_sb = pp.tile([128, NT * 2 * F], BF16, tag="s_sb")


            def rsqrt_row(dst, src, width, tmp_pool):
                """dst = 1/sqrt(src) elementwise on a [1, width] f32 row (DVE only)."""
                yi = tmp_pool.tile([1, width], I32, tag="rsq_i")
                nc.vector.tensor_scalar(
                    out=yi[:], in0=src.bitcast(I32), scalar1=1, scalar2=None,
                    op0=ALU.logical_shift_right)
                nc.vector.tensor_scalar(
                    out=yi[:], in0=yi[:], scalar1=-1, scalar2=0x5f3759df,
                    op0=ALU.mult, op1=ALU.add)
                y = yi[:].bitcast(F32)
                t = tmp_pool.tile([1, width], F32, tag="rsq_t")
                for _ in range(4):
                    nc.vector.tensor_mul(t[:], y, y)
                    nc.vector.tensor_mul(t[:], t[:], src)
                    nc.vector.tensor_scalar(
                        out=t[:], in0=t[:], scalar1=-0.5, scalar2=1.5,
                        op0=ALU.mult, op1=ALU.add)
                    nc.vector.tensor_mul(y, y, t[:])
                nc.vector.tensor_copy(dst, y)


            def sigmoid_sep(out, x):
                nc.scalar.activation(out, x, AF.Abs)
                nc.scalar.activation(out, out, AF.Exp, scale=-1.0)
                nc.scalar.activation(out, out, AF.Ln, bias=1.0)
                nc.vector.scalar_tensor_tensor(
                    out=out, in0=x, scalar=0.0, in1=out,
                    op0=ALU.min, op1=ALU.subtract)
                nc.scalar.activation(out, out, AF.Exp)

            def softplus_sep(out, x):
                """out = softplus(x); out must not alias x (used as scratch)."""
                nc.scalar.activation(out, x, AF.Abs)
                nc.scalar.activation(out, out, AF.Exp, scale=-1.0)
                nc.scalar.activation(out, out, AF.Ln, bias=1.0)
                nc.vector.scalar_tensor_tensor(
                    out=out, in0=x, scalar=0.0, in1=out,
                    op0=ALU.max, op1=ALU.add)

            # ---- embedding ----
            wemb_sb = sp.tile([ORIG + 1, F], F32, tag="emb_w")
            nc.sync.dma_start(wemb_sb[:], wemb[:])
            with tc.tile_pool(name="ps_emb", bufs=2, space="PSUM") as pse:
                for t, (o, L) in enumerate(TILES):
                    lhs = wp.tile([ORIG + 1, 128], F32, tag="emb_lhs")
                    nc.sync.dma_start(lhs[:, :L], atomT[:, o:o + L])
                    ps = pse.tile([128, F], F32, tag="emb_ps")
                    nc.tensor.matmul(ps[:L, :], lhs[:, :L], wemb_sb[:],
                                     start=True, stop=True)
                    nc.vector.tensor_copy(a_sb[:L, t * F:(t + 1) * F], ps[:L, :])

            def conv_layer(l, ain_sb, aout_sb):
                # ---- phase 0: s = a @ W_self + b, a -> bf16 table, AllGather ----
                wsl = sp.tile([F + 1, 2 * F], BF16, tag="wsl")
                wslf = wp.tile([F + 1, 2 * F], F32, tag="wslf")
                nc.sync.dma_start(wslf[:], wself[l])
                nc.vector.tensor_copy(wsl[:], wslf[:])
                wnb0 = sp.tile([F, 2 * F], BF16, tag="wnb")
                nc.sync.dma_start(wnb0[:], wnbr[l])
                with tc.tile_pool(name=f"ps0_{l}", bufs=2, space="PSUM") as ps0:
                    for t, (o, L) in enumerate(TILES):
                        at = ain_sb[:L, t * F:(t + 1) * F]
                        ab = wp.tile([128, F], BF16, tag="ph0ab")
                        nc.vector.tensor_copy(ab[:L, :], at)
                        ps = ps0.tile([128, 128], BF16, tag="tps")
                        nc.tensor.transpose(ps[:F, :L], ab[:L, :], ident_b[:L, :L])
                        lhs = wp.tile([F + 1, 128], BF16, tag="ph0lhs")
                        nc.vector.tensor_copy(lhs[:F, :L], ps[:F, :L])
                        nc.vector.memset(lhs[F:F + 1, :], 1.0)
                        ps2 = ps0.tile([128, 2 * F], F32, tag="sps")
                        nc.tensor.matmul(ps2[:L, :], lhs[:, :L], wsl[:],
                                         start=True, stop=True)
                        nc.vector.tensor_copy(s_sb[:L, t * 2 * F:(t + 1) * 2 * F],
                                              ps2[:L, :])
                        ps3 = ps0.tile([128, 2 * F], F32, tag="pps")
                        nc.tensor.matmul(ps3[:L, :], lhs[:F, :L], wnb0[:],
                                         start=True, stop=True)
                        pb = wp.tile([128, 2 * F], BF16, tag="pb")
                        nc.vector.tensor_copy(pb[:L, :], ps3[:L, :])
                        nc.sync.dma_start(ag_in[o:o + L, :], pb[:L, :])

                nc.gpsimd.collective_compute(
                    "AllGather", ALU.bypass, replica_groups=RG,
                    ins=[ag_in[:]], outs=[a_full[:]])

                wnf = sp.tile([NBR, 2 * F], BF16, tag="wnf")
                nc.sync.dma_start(wnf[:], wnbrf[l])

                # ---- pass A ----
                with tc.tile_pool(name=f"psA_{l}", bufs=2, space="PSUM") as psA, \
                     tc.tile_pool(name=f"psS_{l}", bufs=1, space="PSUM") as psS:
                    st_ps = psS.tile([1, 512], F32, tag="st_x")
                    st2_ps = psS.tile([1, 512], F32, tag="st_x2")
                    for t, (o, L) in enumerate(TILES):
                        idx_t = wp.tile([128, M], I32, tag="idx_t")
                        nc.sync.dma_start(idx_t[:L, :], idx[o:o + L, :])
                        nt = wp.tile([NBR, M * 128], BF16, tag="nbrt")
                        nc.sync.dma_start(nt[:], nbrT[t])
                        gated = wp.tile([128, M * 2 * F], BF16, tag="gated")
                        for mq in range(NQ):
                            pq = psA.tile([128, 512], F32, tag="pqps")
                            for mi in range(4):
                                m = mq * 4 + mi
                                g = wp.tile([128, 2 * F], BF16, tag="g")
                                nc.gpsimd.indirect_dma_start(
                                    out=g[:L, :], out_offset=None, in_=a_full[:],
                                    in_offset=IndirectOffsetOnAxis(
                                        ap=idx_t[:L, m:m + 1], axis=0))
                                sl = pq[:L, mi * 2 * F:(mi + 1) * 2 * F]
                                nc.tensor.matmul(sl, nt[:, m * 128:m * 128 + L],
                                                 wnf[:], start=True, stop=True)
                                # gated = q + p_gathered, then + s
                                nc.vector.tensor_tensor(
                                    out=gated[:L, m * 2 * F:(m + 1) * 2 * F],
                                    in0=sl, in1=g[:L, :], op=ALU.add)
                                nc.vector.tensor_tensor(
                                    out=gated[:L, m * 2 * F:(m + 1) * 2 * F],
                                    in0=gated[:L, m * 2 * F:(m + 1) * 2 * F],
                                    in1=s_sb[:L, t * 2 * F:(t + 1) * 2 * F],
                                    op=ALU.add)
                            q = mq
                            nc.tensor.matmul(
                                st_ps[:, :], ones_col_b[:L, :],
                                gated[:L, q * 512:(q + 1) * 512],
                                start=(t == 0 and q == 0),
                                stop=(t == NT - 1 and q == NQ - 1))
                            g2 = wp.tile([128, 512], F32, tag="g2")
                            nc.scalar.square(g2[:L, :],
                                             gated[:L, q * 512:(q + 1) * 512])
                            nc.tensor.matmul(
                                st2_ps[:, :], ones_col[:L, :], g2[:L, :],
                                start=(t == 0 and q == 0),
                                stop=(t == NT - 1 and q == NQ - 1))
                        nc.sync.dma_start(stash[t, :L, :], gated[:L, :])

                    str_sb = sp.tile([1, 1024], F32, tag="str")
                    nc.vector.tensor_copy(str_sb[:, :512], st_ps[:])
                    nc.vector.tensor_copy(str_sb[:, 512:], st2_ps[:])

                nc.sync.dma_start(cc_in[:], str_sb[:])
                nc.gpsimd.collective_compute(
                    "AllReduce", ALU.add, replica_groups=RG,
                    ins=[cc_in[:]], outs=[cc_out[:]])
                stg = sp.tile([1, 1024], F32, tag="str")
                nc.sync.dma_start(stg[:], cc_out[:])

                # fold the 4 column-groups, build affine rows
                mean = sp.tile([1, 2 * F], F32, tag="mean")
                nc.vector.reduce_sum(
                    mean[:], stg[:, :512].rearrange("p (g c) -> p c g", g=4),
                    axis=mybir.AxisListType.X)
                nc.vector.tensor_scalar_mul(mean[:], mean[:], 1.0 / (N * M))
                ex2 = sp.tile([1, 2 * F], F32, tag="ex2")
                nc.vector.reduce_sum(
                    ex2[:], stg[:, 512:].rearrange("p (g c) -> p c g", g=4),
                    axis=mybir.AxisListType.X)
                nc.vector.tensor_scalar_mul(ex2[:], ex2[:], 1.0 / (N * M))
                var = sp.tile([1, 2 * F], F32, tag="var")
                nc.vector.tensor_mul(var[:], mean[:], mean[:])
                nc.vector.tensor_sub(var[:], ex2[:], var[:])
                nc.vector.tensor_scalar_add(var[:], var[:], EPS)
                rstd = sp.tile([1, 2 * F], F32, tag="rstd")
                rsqrt_row(rstd[:], var[:], 2 * F, sp)
                g1 = sp.tile([1, 2 * F], F32, tag="g1r")
                nc.sync.dma_start(g1[:], bn1g[l])
                b1 = sp.tile([1, 2 * F], F32, tag="b1r")
                nc.sync.dma_start(b1[:], bn1b[l])
                sc_row = sp.tile([1, 2 * F], F32, tag="sc_row")
                nc.vector.tensor_mul(sc_row[:], rstd[:], g1[:])
                bi_row = sp.tile([1, 2 * F], F32, tag="bi_row")
                nc.vector.tensor_mul(bi_row[:], mean[:], sc_row[:])
                nc.vector.tensor_sub(bi_row[:], b1[:], bi_row[:])

                scb = pp.tile([128, 2 * F], F32, tag="scb")
                bib = pp.tile([128, 2 * F], F32, tag="bib")
                with tc.tile_pool(name=f"psR_{l}", bufs=1, space="PSUM") as psR:
                    rp = psR.tile([128, 2 * F], F32, tag="rowps")
                    nc.tensor.matmul(rp[:], ones_row[:1, :], sc_row[:],
                                     start=True, stop=True)
                    nc.vector.tensor_copy(scb[:], rp[:])
                    rp2 = psR.tile([128, 2 * F], F32, tag="rowps2")
                    nc.tensor.matmul(rp2[:], ones_row[:1, :], bi_row[:],
                                     start=True, stop=True)
                    nc.vector.tensor_copy(bib[:], rp2[:])

                # ---- pass B ----
                with tc.tile_pool(name=f"psB_{l}", bufs=1, space="PSUM") as psB:
                    s2_ps = psB.tile([1, 2 * F], F32, tag="s2ps")
                    for t, (o, L) in enumerate(TILES):
                        gated = wp.tile([128, M * 2 * F], BF16, tag="gatedB")
                        nc.sync.dma_start(gated[:L, :], stash[t, :L, :])
                        u = wp.tile([128, M * 2 * F], F32, tag="u")
                        for m in range(M):
                            msl = slice(m * 2 * F, (m + 1) * 2 * F)
                            nc.vector.tensor_mul(u[:L, msl], gated[:L, msl],
                                                 scb[:L, :])
                            nc.vector.tensor_add(u[:L, msl], u[:L, msl],
                                                 bib[:L, :])
                        uv = u[:L, :].rearrange("p (m c) -> p m c", m=M)
                        fi = wp.tile([128, M * F], F32, tag="fi")
                        sigmoid_sep(
                            fi[:L, :].rearrange("p (m c) -> p m c", m=M),
                            uv[:, :, :F])
                        co = wp.tile([128, M * F], F32, tag="co")
                        softplus_sep(
                            co[:L, :].rearrange("p (m c) -> p m c", m=M),
                            uv[:, :, F:])
                        nc.vector.tensor_mul(fi[:L, :], fi[:L, :], co[:L, :])
                        sm = sum_sb[:L, t * F:(t + 1) * F]
                        nc.vector.reduce_sum(
                            sm, fi[:L, :].rearrange("p (m c) -> p c m", m=M),
                            axis=mybir.AxisListType.X)
                        s2d = wp.tile([128, F], F32, tag="s2d")
                        nc.scalar.square(s2d[:L, :], sm)
                        nc.tensor.matmul(s2_ps[:, :F], ones_col[:L, :], sm,
                                         start=(t == 0), stop=False)
                        nc.tensor.matmul(s2_ps[:, F:], ones_col[:L, :], s2d[:L, :],
                                         start=(t == 0), stop=(t == NT - 1))
                    st2 = sp.tile([1, 2 * F], F32, tag="st2sb")
                    nc.vector.tensor_copy(st2[:], s2_ps[:])

                nc.sync.dma_start(cc2_in[:], st2[:])
                nc.gpsimd.collective_compute(
                    "AllReduce", ALU.add, replica_groups=RG,
                    ins=[cc2_in[:]], outs=[cc2_out[:]])
                stg2 = sp.tile([1, 2 * F], F32, tag="stg2")
                nc.sync.dma_start(stg2[:], cc2_out[:])
                mean2 = sp.tile([1, F], F32, tag="mean2")
                nc.vector.tensor_scalar_mul(mean2[:], stg2[:, :F], 1.0 / N)
                ex22 = sp.tile([1, F], F32, tag="ex22")
                nc.vector.tensor_scalar_mul(ex22[:], stg2[:, F:], 1.0 / N)
                var2 = sp.tile([1, F], F32, tag="var2")
                nc.vector.tensor_mul(var2[:], mean2[:], mean2[:])
                nc.vector.tensor_sub(var2[:], ex22[:], var2[:])
                nc.vector.tensor_scalar_add(var2[:], var2[:], EPS)
                rstd2 = sp.tile([1, F], F32, tag="rstd2")
                rsqrt_row(rstd2[:], var2[:], F, sp)
                g2r = sp.tile([1, F], F32, tag="g2rr")
                nc.sync.dma_start(g2r[:], bn2g[l])
                b2r = sp.tile([1, F], F32, tag="b2rr")
                nc.sync.dma_start(b2r[:], bn2b[l])
                sc2 = sp.tile([1, F], F32, tag="sc2")
                nc.vector.tensor_mul(sc2[:], rstd2[:], g2r[:])
                bi2 = sp.tile([1, F], F32, tag="bi2")
                nc.vector.tensor_mul(bi2[:], mean2[:], sc2[:])
                nc.vector.tensor_sub(bi2[:], b2r[:], bi2[:])
                sc2b = pp.tile([128, F], F32, tag="sc2b")
                bi2b = pp.tile([128, F], F32, tag="bi2b")
                with tc.tile_pool(name=f"psR2_{l}", bufs=1, space="PSUM") as psR2:
                    rp = psR2.tile([128, F], F32, tag="rowps")
                    nc.tensor.matmul(rp[:], ones_row[:1, :], sc2[:],
                                     start=True, stop=True)
                    nc.vector.tensor_copy(sc2b[:], rp[:])
                    rp2 = psR2.tile([128, F], F32, tag="rowps2")
                    nc.tensor.matmul(rp2[:], ones_row[:1, :], bi2[:],
                                     start=True, stop=True)
                    nc.vector.tensor_copy(bi2b[:], rp2[:])

                for t, (o, L) in enumerate(TILES):
                    sm = sum_sb[:L, t * F:(t + 1) * F]
                    v = wp.tile([128, F], F32, tag="vup")
                    nc.vector.tensor_mul(v[:L, :], sm, sc2b[:L, :])
                    nc.vector.tensor_add(v[:L, :], v[:L, :], bi2b[:L, :])
                    nc.vector.tensor_add(v[:L, :], v[:L, :],
                                         ain_sb[:L, t * F:(t + 1) * F])
                    softplus_sep(aout_sb[:L, t * F:(t + 1) * F], v[:L, :])

            conv_layer(0, a_sb, a1_sb)
            conv_layer(1, a1_sb, a_sb)  # reuse a_sb as conv2 output
            for t, (o, L) in enumerate(TILES):
                sl = slice(t * F, (t + 1) * F)
                vr = wp.tile([128, F], F32, tag="vup")
                nc.vector.tensor_add(vr[:L, :], a_sb[:L, sl], a1_sb[:L, sl])
                softplus_sep(a_sb[:L, sl], vr[:L, :])

            # ---- pooling ----
            iota_sb = sp.tile([128, CLOC], mybir.dt.int16, tag="iota")
            nc.gpsimd.iota(iota_sb[:], pattern=[[1, CLOC]], base=0,
                           channel_multiplier=0)

            with tc.tile_pool(name="psP", bufs=1, space="PSUM") as psP:
                cr_ps = psP.tile([F, CLOC], F32, tag="cr_ps")
                for t, (o, L) in enumerate(TILES):
                    cidt = wp.tile([128, 1], I32, tag="cidt")
                    nc.sync.dma_start(cidt[:L, :], cidl[o:o + L, :])
                    cidf = wp.tile([128, 1], mybir.dt.int16, tag="cidf")
                    nc.vector.tensor_copy(cidf[:L, :], cidt[:L, :])
                    A = wp.tile([128, CLOC], BF16, tag="Amat")
                    nc.vector.tensor_tensor(
                        out=A[:L, :], in0=iota_sb[:L, :],
                        in1=cidf[:L, :].to_broadcast([L, CLOC]),
                        op=ALU.is_equal)
                    a2b = wp.tile([128, F], BF16, tag="a2b")
                    nc.vector.tensor_copy(a2b[:L, :], a_sb[:L, t * F:(t + 1) * F])
                    nc.tensor.matmul(cr_ps[:], a2b[:L, :], A[:L, :],
                                     start=(t == 0), stop=(t == NT - 1))
                crl = wp.tile([F, CLOC], F32, tag="crl")
                nc.vector.tensor_copy(crl[:], cr_ps[:])
            nc.sync.dma_start(cr_in[:], crl[:])
            nc.gpsimd.collective_compute(
                "AllGather", ALU.bypass, replica_groups=RG,
                ins=[cr_in[:]], outs=[cr_out[:]])

            cg = pp.tile([F, N0P], F32, tag="cg")
            nc.vector.memset(cg[:], 0.0)
            for k in range(NC):
                w = wp.tile([F, CLOC], F32, tag="agw")
                nc.sync.dma_start(w[:], cr_out[k])
                nc.vector.tensor_add(cg[:, BASES[k]:BASES[k] + CLOC],
                                     cg[:, BASES[k]:BASES[k] + CLOC], w[:])
            with tc.tile_pool(name="psH", bufs=2, space="PSUM") as psH:
                for q in range(N0P // 512):
                    icr = wp.tile([1, 512], F32, tag="icr")
                    nc.sync.dma_start(icr[:], invc[:, q * 512:(q + 1) * 512])
                    icb = psH.tile([128, 512], F32, tag="icb")
                    nc.tensor.matmul(icb[:F, :], ones_row[:1, :F], icr[:],
                                     start=True, stop=True)
                    tmpc = wp.tile([F, CLOC], F32, tag="agw")
                    nc.vector.tensor_mul(tmpc[:],
                                         cg[:, q * 512:(q + 1) * 512], icb[:F, :])
                    softplus_sep(cg[:, q * 512:(q + 1) * 512], tmpc[:])

                wfc_sb = sp.tile([F + 1, H], F32, tag="wfc_sb")
                nc.sync.dma_start(wfc_sb[:], wfc[:])
                wout_sb = sp.tile([H, 1], F32, tag="wout_sb")
                nc.sync.dma_start(wout_sb[:], wout[:])
                bout_sb = sp.tile([1, 1], F32, tag="bout_sb")
                nc.sync.dma_start(bout_sb[:], bout[:])
                orow = pp.tile([1, N0P], F32, tag="orow")
                rhs = sp.tile([F + 1, 512], F32, tag="head_rhs")
                nc.vector.memset(rhs[F:F + 1, :], 1.0)
                for q in range(N0P // 512):
                    nc.vector.tensor_copy(rhs[:F, :], cg[:, q * 512:(q + 1) * 512])
                    h_ps = psH.tile([128, 512], F32, tag="h_ps")
                    nc.tensor.matmul(h_ps[:H, :], wfc_sb[:], rhs[:],
                                     start=True, stop=True)
                    h_sb = wp.tile([H, 512], F32, tag="h_sb")
                    softplus_sep(h_sb[:], h_ps[:H, :])
                    o_ps = psH.tile([128, 512], F32, tag="o_ps")
                    nc.tensor.matmul(o_ps[:1, :], wout_sb[:], h_sb[:],
                                     start=True, stop=True)
                    nc.scalar.activation(orow[:, q * 512:(q + 1) * 512],
                                         o_ps[:1, :], AF.Identity,
                                         bias=bout_sb[:, :1])
            nc.sync.dma_start(out[:], orow[:, :N0])

    split_sync_waits(nc)
    return nc


_prog_cache = {}


def kernel(**inputs):
    atom_fea = np.asarray(inputs["atom_fea"], np.float32)
    nbr_fea = np.asarray(inputs["nbr_fea"], np.float32)
    nbr_fea_idx = np.asarray(inputs["nbr_fea_idx"]).astype(np.int32)
    crystal_id = np.asarray(inputs["crystal_id"]).astype(np.int32)

    W_emb = np.asarray(inputs["W_emb"], np.float32)
    b_emb = np.asarray(inputs["b_emb"], np.float32)
    wemb93 = np.vstack([W_emb, b_emb[None, :]]).astype(np.float32)

    def layer_w(Wn, bn):
        W = np.asarray(Wn, np.float32)
        b = np.asarray(bn, np.float32)
        return (np.vstack([W[:F], b[None, :]]).astype(np.float32),
                np.ascontiguousarray(W[F:2 * F]).astype(ml_dtypes.bfloat16),
                np.ascontiguousarray(W[2 * F:]).astype(ml_dtypes.bfloat16))

    ws0, wn0, wf0 = layer_w(inputs["cW"], inputs["cb"])
    ws1, wn1, wf1 = layer_w(inputs["rW"], inputs["rb"])
    pack = lambda a, b: np.ascontiguousarray(np.stack([a, b]))
    wself_np, wnbr_np, wnbrf_np = pack(ws0, ws1), pack(wn0, wn1), pack(wf0, wf1)
    r1 = lambda k: np.asarray(inputs[k], np.float32)[None, :]
    bn1g_np, bn1b_np = pack(r1("cg1"), r1("rg1")), pack(r1("cbt1"), r1("rbt1"))
    bn2g_np, bn2b_np = pack(r1("cg2"), r1("rg2")), pack(r1("cbt2"), r1("rbt2"))
    wfc_np = np.vstack([np.asarray(inputs["W_fc"], np.float32),
                        np.asarray(inputs["b_fc"], np.float32)[None, :]])
    wout_np = np.asarray(inputs["W_out"], np.float32)
    bout_np = np.asarray(inputs["b_out"], np.float32).reshape(1, 1)

    cnt = np.bincount(crystal_id, minlength=N0).astype(np.float32)
    inv = (1.0 / np.maximum(cnt, 1.0)).astype(np.float32)
    invc_np = np.zeros((1, N0P), np.float32)
    invc_np[0, :N0] = inv

    nbrT_b = nbr_fea.astype(ml_dtypes.bfloat16)
    atomT_full = np.vstack([atom_fea.T, np.ones((1, N), np.float32)])

    def pack_nbr(nb, lo):
        outp = np.zeros((NT, NBR, M, 128), ml_dtypes.bfloat16)
        for t, (o, L) in enumerate(TILES):
            blk = nb[lo + o:lo + o + L]            # [L, M, NBR]
            outp[t, :, :, :L] = np.transpose(blk, (2, 1, 0))
        return outp.reshape(NT, NBR, M * 128)

    in_maps = []
    for c in range(NC):
        lo, hi = c * S, (c + 1) * S
        cidl_np = (crystal_id[lo:hi] - BASES[c]).astype(np.int32)
        assert cidl_np.min() >= 0 and cidl_np.max() < CLOC, (
            f"core {c}: crystal window [{cidl_np.min()},{cidl_np.max()}]")
        in_maps.append({
            "atomT": np.ascontiguousarray(atomT_full[:, lo:hi]),
            "nbrT": pack_nbr(nbrT_b, lo),
            "idx": np.ascontiguousarray(nbr_fea_idx[lo:hi]),
            "cidl": cidl_np.reshape(S, 1),
            "invc": invc_np,
            "wemb": wemb93, "wself": wself_np, "wnbr": wnbr_np,
            "wnbrf": wnbrf_np, "bn1g": bn1g_np, "bn1b": bn1b_np,
            "bn2g": bn2g_np, "bn2b": bn2b_np,
            "wfc": wfc_np, "wout": wout_np, "bout": bout_np,
        })

    if "prog" not in _prog_cache:
        _prog_cache["prog"] = build_program()
    nc = _prog_cache["prog"]

    trace = bool(int(os.environ.get("KERNEL_TRACE", "0")))
    res = run_bass_kernel_spmd(nc, in_maps, core_ids=list(range(NC)), trace=trace)
    if trace:
        kernel.last_exec_ns = res.exec_time_ns
        kernel.last_trace = (res.instructions_and_trace or (None, None))[1]
    return res.results[0]["out"].reshape(N0, 1).astype(np.float32)

